# revision 14
# baseline (speedup 1.0000x reference)
"""CrossPlaneMixer Trainium2 kernel.

Problem: three 5D "plane" tensors (B=2, C=64) at mixed resolutions:
  dh: [2,64,64,64,16]  (full D,H; small W)
  dw: [2,64,64,16,64]  (full D,W; small H)
  hw: [2,64,16,64,64]  (full H,W; small D)
Each plane is mean-reduced along its small axis, the summaries are pooled +
broadcast into the other planes' grids, concatenated on channels (3C=192),
and mixed by a 1x1x1 conv (w: [64,192], bias [64]).

Distribution: 8 NeuronCores. dh/dw sharded over D (8 slices of 8), hw over
H.  SBUF partition dim = (b=2 x c=64) = 128 everywhere; a block-diagonal
[128,128] weight (the 64x64 block replicated on both halves) makes a single
full-array K=128 matmul cover both batch items.

Per output tile the conv is 3 accumulating PSUM matmuls:
  term1: the plane itself           (rhs = streamed bf16 input tile)
  term2/term3: pooled summaries     (rhs = small SBUF tables, broadcast via
                                     stride-0 AP dims — no materialization)
Matmul operands are bf16 (fp32 would be 4 cyc/row and double the DMA bytes);
accumulation stays fp32 in PSUM.  Eviction PSUM->SBUF runs on VectorE as
tensor_scalar_add(bias) with bf16 output; outputs are upcast to fp32 on the
host.  Input DMAs issue on the SP HWDGE queue, output DMAs on the ACT HWDGE
queue so a blocked output never stalls the input stream.

The small pooled summaries (<0.3% of reference FLOPs) are precomputed on the
host and passed per-core, so cores need no cross-core communication.
"""

import os
from contextlib import ExitStack

import ml_dtypes
import numpy as np

import concourse.bass as bass
import concourse.tile as tile
from concourse import bacc, mybir
import concourse.bass_utils as bass_utils

B, C, FULL, SMALL = 2, 64, 64, 16
NCORES = 8
DSH = FULL // NCORES  # 8: per-core slice of D (dh,dw) or H (hw)
P = B * C  # 128 partitions = (b, c)

F32 = mybir.dt.float32
BF16 = mybir.dt.bfloat16
NPBF16 = ml_dtypes.bfloat16

NCH = 512  # free elems per matmul = one fp32 PSUM bank

# packed const layout (bf16 elems per partition)
OFF_W = 0                       # 9 x 128 block-diag transposed weights
OFF_L1 = OFF_W + 9 * P          # poolW(sum_dw)[d_loc, w16]
OFF_L2 = OFF_L1 + DSH * SMALL   # poolH(sum_dh)[d_loc, h16]
OFF_G1 = OFF_L2 + DSH * SMALL   # poolW(sum_hw)[h, w16]
OFF_G2 = OFF_G1 + FULL * SMALL  # poolH(sum_hw)[h16, w]
OFF_G3 = OFF_G2 + SMALL * FULL  # poolD(sum_dh)[d16, h_loc]
OFF_G4 = OFF_G3 + SMALL * DSH   # poolD(sum_dw)[d16, w]
OFF_B = OFF_G4 + SMALL * FULL   # bias (bf16; upconverted on device)
CLEN = OFF_B + 3

_BUILT = None
LAST_RESULTS = None  # BassKernelResults of the most recent run (for test.py)


def _build():
    nc = bacc.Bacc(
        "TRN2",
        target_bir_lowering=False,
        debug=False,
        enable_asserts=False,
        num_devices=NCORES,
    )

    # ---- per-core DRAM I/O (bf16 on the wire) ----
    x_dh = nc.dram_tensor("x_dh", [B, C, DSH, FULL, SMALL], BF16, kind="ExternalInput")
    x_dw = nc.dram_tensor("x_dw", [B, C, DSH, SMALL, FULL], BF16, kind="ExternalInput")
    x_hw = nc.dram_tensor("x_hw", [B, C, SMALL, DSH, FULL], BF16, kind="ExternalInput")
    # one packed bf16 const tensor per core: block-diag transposed weights,
    # pooled summary tables, bf16 bias (see _CST_* offsets)
    cst = nc.dram_tensor("cst", [P, CLEN], BF16, kind="ExternalInput")

    y_dh = nc.dram_tensor("y_dh", [B, C, DSH, FULL, SMALL], BF16, kind="ExternalOutput")
    y_dw = nc.dram_tensor("y_dw", [B, C, DSH, SMALL, FULL], BF16, kind="ExternalOutput")
    y_hw = nc.dram_tensor("y_hw", [B, C, SMALL, DSH, FULL], BF16, kind="ExternalOutput")

    with tile.TileContext(nc) as tc, ExitStack() as ctx:
        cpool = ctx.enter_context(tc.tile_pool(name="const", bufs=1))
        inp = ctx.enter_context(tc.tile_pool(name="inp", bufs=4))
        psp = ctx.enter_context(tc.tile_pool(name="psum", bufs=2, space="PSUM"))
        outp = ctx.enter_context(tc.tile_pool(name="outp", bufs=4))

        # ---- all constants in ONE packed DMA (contiguous lines) ----
        csb = cpool.tile([P, CLEN], BF16)
        nc.sync.dma_start(csb[:], cst.ap())
        l1sb = csb[:, OFF_L1 : OFF_L1 + DSH * SMALL].rearrange(
            "p (a b) -> p a b", a=DSH
        )
        l2sb = csb[:, OFF_L2 : OFF_L2 + DSH * SMALL].rearrange(
            "p (a b) -> p a b", a=DSH
        )
        g1sb = csb[:, OFF_G1 : OFF_G1 + FULL * SMALL].rearrange(
            "p (a b) -> p a b", a=FULL
        )
        g2sb = csb[:, OFF_G2 : OFF_G2 + SMALL * FULL].rearrange(
            "p (a b) -> p a b", a=SMALL
        )
        g3sb = csb[:, OFF_G3 : OFF_G3 + SMALL * DSH].rearrange(
            "p (a b) -> p a b", a=SMALL
        )
        g4sb = csb[:, OFF_G4 : OFF_G4 + SMALL * FULL].rearrange(
            "p (a b) -> p a b", a=SMALL
        )
        bsb = cpool.tile([P, 3], F32)
        nc.vector.tensor_copy(bsb[:], csb[:, OFF_B : OFF_B + 3])

        def w_of(plane, s):
            i = (3 * plane + s) * P
            return csb[:, i : i + P]

        def evict_half(plane, ps_ap, osb_ap, on_act):
            # PSUM -> SBUF with per-channel bias, fp32 -> bf16.
            if on_act:
                nc.scalar.activation(
                    osb_ap, ps_ap, mybir.ActivationFunctionType.Identity,
                    bias=bsb[:, plane : plane + 1],
                )
            else:
                nc.vector.tensor_scalar_add(osb_ap, ps_ap, bsb[:, plane : plane + 1])

        # ---------- planes 0/1: dh, dw (identical structure) ----------
        # in tile = d-quad [128, 4, 1024] (8KB contiguous lines); 2 per plane
        for plane, x, y in ((0, x_dh, y_dh), (1, x_dw, y_dw)):
            xv = x.ap().rearrange("b c d h w -> (b c) d (h w)")
            yv = y.ap().rearrange("b c d h w -> (b c) (d h w)")
            for q in range(DSH // 4):
                tin = inp.tile([P, 4, 1024], BF16, tag="in")
                if plane == 0 and q == 0:
                    # split the very first load so compute starts ~1.5us earlier
                    nc.sync.dma_start(tin[:, 0:2, :], xv[:, 0:2, :])
                    nc.sync.dma_start(tin[:, 2:4, :], xv[:, 2:4, :])
                else:
                    nc.sync.dma_start(tin[:], xv[:, 4 * q : 4 * q + 4, :])
                for t in range(2):  # psum tile per d-pair
                    ps = psp.tile([P, 2048], F32)
                    for s in range(3):
                        lhsT = w_of(plane, s)
                        for j in range(2):  # d within pair
                            dl = 4 * q + 2 * t + j
                            for n in range(2):  # 512-chunk
                                if s == 0:
                                    rhs = tin[:, 2 * t + j, NCH * n : NCH * (n + 1)]
                                elif s == 1:
                                    if plane == 0:
                                        # l1[dl, w16] bcast over h (chunk rows)
                                        rhs = l1sb[:, dl : dl + 1, :].broadcast_to(
                                            [P, 32, SMALL]
                                        )
                                    else:
                                        # l2[dl, h16] chunk, bcast over w
                                        rhs = (
                                            l2sb[:, dl, 8 * n : 8 * (n + 1)]
                                            .unsqueeze(2)
                                            .broadcast_to([P, 8, FULL])
                                        )
                                elif plane == 0:
                                    rhs = g1sb[:, 32 * n : 32 * (n + 1), :]
                                else:
                                    rhs = g2sb[:, 8 * n : 8 * (n + 1), :]
                                nc.tensor.matmul(
                                    ps[:, (j * 2 + n) * NCH : (j * 2 + n + 1) * NCH],
                                    lhsT,
                                    rhs,
                                    start=(s == 0),
                                    stop=(s == 2),
                                )
                    osb = outp.tile([P, 2048], BF16, tag="out")
                    evict_half(plane, ps[:], osb[:], on_act=False)
                    nc.scalar.dma_start(
                        yv[:, (4 * q + 2 * t) * 1024 : (4 * q + 2 * t + 2) * 1024],
                        osb[:],
                    )

        # ---------- plane 2: hw ----------
        # in tile = h-quad [128, 16, 4, 64] (512B lines); 2 tiles
        xv = x_hw.ap().rearrange("b c d h w -> (b c) d h w")
        yv = y_hw.ap().rearrange("b c d h w -> (b c) d h w")
        for q in range(DSH // 4):
            tin = inp.tile([P, SMALL, 4, FULL], BF16, tag="in")
            nc.sync.dma_start(tin[:], xv[:, :, 4 * q : 4 * q + 4, :])
            for t in range(2):  # psum tile per 8 d's
                ps = psp.tile([P, 2048], F32)
                for s in range(3):
                    lhsT = w_of(2, s)
                    for m in range(4):  # chunk: d in [8t+2m, 8t+2m+2)
                        d0 = 8 * t + 2 * m
                        if s == 0:
                            rhs = tin[:, d0 : d0 + 2, :, :]
                        elif s == 1:
                            # g3[d16, h_loc] chunk, bcast over w
                            rhs = (
                                g3sb[:, d0 : d0 + 2, 4 * q : 4 * q + 4]
                                .unsqueeze(3)
                                .broadcast_to([P, 2, 4, FULL])
                            )
                        else:
                            # g4[d16, w] chunk, bcast over h (middle)
                            rhs = (
                                g4sb[:, d0 : d0 + 2, :]
                                .unsqueeze(2)
                                .broadcast_to([P, 2, 4, FULL])
                            )
                        nc.tensor.matmul(
                            ps[:, m * NCH : (m + 1) * NCH],
                            lhsT,
                            rhs,
                            start=(s == 0),
                            stop=(s == 2),
                        )
                osb = outp.tile([P, 8, 4, FULL], BF16, tag="out")
                osb_flat = osb[:].rearrange("p a b c -> p (a b c)")
                if q == DSH // 4 - 1 and t == 1:
                    # final tile: halves on DVE||ACT + 2 DMAs to cut the tail
                    for j in range(2):
                        h = slice(j * 1024, (j + 1) * 1024)
                        evict_half(2, ps[:, h], osb_flat[:, h], on_act=(j == 1))
                        nc.scalar.dma_start(
                            yv[:, 8 * t + 4 * j : 8 * t + 4 * (j + 1), 4 * q : 4 * q + 4, :],
                            osb[:, 4 * j : 4 * (j + 1)],
                        )
                else:
                    evict_half(2, ps[:], osb_flat, on_act=False)
                    nc.scalar.dma_start(
                        yv[:, 8 * t : 8 * t + 8, 4 * q : 4 * q + 4, :], osb[:]
                    )

    nc.compile()
    return nc


def _pool4(x, axis):
    # exact adaptive mean-pool by 4 along `axis` (64 -> 16)
    shp = list(x.shape)
    shp[axis] = 16
    shp.insert(axis + 1, 4)
    return x.reshape(shp).mean(axis=axis + 1)


def _prep_inputs(dh, dw, hw, w_dh, b_dh, w_dw, b_dw, w_hw, b_hw):
    f32 = np.float32
    dh, dw, hw = (np.ascontiguousarray(a, f32) for a in (dh, dw, hw))

    sum_dh = dh.mean(axis=4)  # [b,c,d,h]
    sum_dw = dw.mean(axis=3)  # [b,c,d,w]
    sum_hw = hw.mean(axis=2)  # [b,c,h,w]

    p_wdw = _pool4(sum_dw, 3)  # [b,c,d,16]   dw_in_dh
    p_hdh = _pool4(sum_dh, 3)  # [b,c,d,16]   dh_in_dw
    p_whw = _pool4(sum_hw, 3)  # [b,c,h,16]   hw_in_dh
    p_hhw = _pool4(sum_hw, 2)  # [b,c,16,w]   hw_in_dw
    p_ddh = _pool4(sum_dh, 2)  # [b,c,16,h]   dh_in_hw
    p_ddw = _pool4(sum_dw, 2)  # [b,c,16,w]   dw_in_hw

    bf = lambda a: np.ascontiguousarray(a, NPBF16)
    dh, dw, hw = bf(dh), bf(dw), bf(hw)

    # packed per-core const tensor [P, CLEN]
    cst0 = np.zeros((P, CLEN), NPBF16)
    for pi, w in enumerate((w_dh, w_dw, w_hw)):
        w = np.asarray(w, f32)
        for s in range(3):
            blk = bf(w[:, 64 * s : 64 * (s + 1)].T)  # [c_in, o]
            i = (3 * pi + s) * P
            cst0[0:64, i : i + 64] = blk
            cst0[64:128, i + 64 : i + 128] = blk
    cst0[:, OFF_B : OFF_B + 3] = bf(
        np.stack(
            [np.concatenate([np.asarray(b, f32)] * 2) for b in (b_dh, b_dw, b_hw)],
            axis=1,
        )
    )
    cst0[:, OFF_G1 : OFF_G1 + FULL * SMALL] = bf(p_whw.reshape(P, FULL * SMALL))
    cst0[:, OFF_G2 : OFF_G2 + SMALL * FULL] = bf(p_hhw.reshape(P, SMALL * FULL))
    cst0[:, OFF_G4 : OFF_G4 + SMALL * FULL] = bf(p_ddw.reshape(P, SMALL * FULL))

    in_maps = []
    for k in range(NCORES):
        dsl = slice(DSH * k, DSH * (k + 1))
        cst = cst0.copy()
        cst[:, OFF_G3 : OFF_G3 + SMALL * DSH] = bf(
            p_ddh.reshape(P, SMALL, FULL)[:, :, dsl].reshape(P, SMALL * DSH)
        )
        cst[:, OFF_L1 : OFF_L1 + DSH * SMALL] = bf(
            p_wdw.reshape(P, FULL, SMALL)[:, dsl, :].reshape(P, DSH * SMALL)
        )
        cst[:, OFF_L2 : OFF_L2 + DSH * SMALL] = bf(
            p_hdh.reshape(P, FULL, SMALL)[:, dsl, :].reshape(P, DSH * SMALL)
        )
        in_maps.append(
            {
                "x_dh": np.ascontiguousarray(dh[:, :, dsl]),
                "x_dw": np.ascontiguousarray(dw[:, :, dsl]),
                "x_hw": np.ascontiguousarray(hw[:, :, :, dsl, :]),
                "cst": cst,
            }
        )
    return in_maps


def _run(inputs: dict, trace: bool = False):
    global _BUILT, LAST_RESULTS
    if _BUILT is None:
        _BUILT = _build()
    nc = _BUILT

    in_maps = _prep_inputs(**inputs)
    res = bass_utils.run_bass_kernel_spmd(
        nc, in_maps, core_ids=list(range(NCORES)), trace=trace
    )
    LAST_RESULTS = res

    dh_new = np.empty((B, C, FULL, FULL, SMALL), np.float32)
    dw_new = np.empty((B, C, FULL, SMALL, FULL), np.float32)
    hw_new = np.empty((B, C, SMALL, FULL, FULL), np.float32)
    for k in range(NCORES):
        dsl = slice(DSH * k, DSH * (k + 1))
        dh_new[:, :, dsl] = res.results[k]["y_dh"].astype(np.float32)
        dw_new[:, :, dsl] = res.results[k]["y_dw"].astype(np.float32)
        hw_new[:, :, :, dsl, :] = res.results[k]["y_hw"].astype(np.float32)
    return dh_new, dw_new, hw_new


def kernel(**inputs):
    return _run(inputs, trace=bool(os.environ.get("KERNEL_TRACE")))


# revision 15
# speedup vs baseline: 1.0570x; 1.0570x over previous
"""CrossPlaneMixer Trainium2 kernel.

Problem: three 5D "plane" tensors (B=2, C=64) at mixed resolutions:
  dh: [2,64,64,64,16]  (full D,H; small W)
  dw: [2,64,64,16,64]  (full D,W; small H)
  hw: [2,64,16,64,64]  (full H,W; small D)
Each plane is mean-reduced along its small axis, the summaries are pooled +
broadcast into the other planes' grids, concatenated on channels (3C=192),
and mixed by a 1x1x1 conv (w: [64,192], bias [64]).

Distribution: 8 NeuronCores. dh/dw sharded over D (8 slices of 8), hw over
H.  SBUF partition dim = (b=2 x c=64) = 128 everywhere; a block-diagonal
[128,128] weight (the 64x64 block replicated on both halves) makes a single
full-array K=128 matmul cover both batch items.

Per output tile the conv is 3 accumulating PSUM matmuls:
  term1: the plane itself           (rhs = streamed bf16 input tile)
  term2/term3: pooled summaries     (rhs = small SBUF tables, broadcast via
                                     stride-0 AP dims — no materialization)
Matmul operands are bf16 (fp32 would be 4 cyc/row and double the DMA bytes);
accumulation stays fp32 in PSUM.  Eviction PSUM->SBUF runs on VectorE as
tensor_scalar_add(bias) with bf16 output; outputs are upcast to fp32 on the
host.  Input DMAs issue on the SP HWDGE queue, output DMAs on the ACT HWDGE
queue so a blocked output never stalls the input stream.

The small pooled summaries (<0.3% of reference FLOPs) are precomputed on the
host and passed per-core, so cores need no cross-core communication.
"""

import os
from contextlib import ExitStack

import ml_dtypes
import numpy as np

import concourse.bass as bass
import concourse.tile as tile
from concourse import bacc, mybir
import concourse.bass_utils as bass_utils

B, C, FULL, SMALL = 2, 64, 64, 16
NCORES = 8
DSH = FULL // NCORES  # 8: per-core slice of D (dh,dw) or H (hw)
P = B * C  # 128 partitions = (b, c)

F32 = mybir.dt.float32
BF16 = mybir.dt.bfloat16
NPBF16 = ml_dtypes.bfloat16

NCH = 512  # free elems per matmul = one fp32 PSUM bank

# packed const layout (bf16 elems per partition)
OFF_W = 0                       # 9 x 128 block-diag transposed weights
OFF_L1 = OFF_W + 9 * P          # poolW(sum_dw)[d_loc, w16]
OFF_L2 = OFF_L1 + DSH * SMALL   # poolH(sum_dh)[d_loc, h16]
OFF_G1 = OFF_L2 + DSH * SMALL   # poolW(sum_hw)[h, w16]
OFF_G2 = OFF_G1 + FULL * SMALL  # poolH(sum_hw)[h16, w]
OFF_G3 = OFF_G2 + SMALL * FULL  # poolD(sum_dh)[d16, h_loc]
OFF_G4 = OFF_G3 + SMALL * DSH   # poolD(sum_dw)[d16, w]
OFF_B = OFF_G4 + SMALL * FULL   # bias (bf16; upconverted on device)
CLEN = OFF_B + 3

_BUILT = None
LAST_RESULTS = None  # BassKernelResults of the most recent run (for test.py)


def _build():
    nc = bacc.Bacc(
        "TRN2",
        target_bir_lowering=False,
        debug=False,
        enable_asserts=False,
        num_devices=NCORES,
    )

    # ---- per-core DRAM I/O (bf16 on the wire) ----
    x_dh = nc.dram_tensor("x_dh", [B, C, DSH, FULL, SMALL], BF16, kind="ExternalInput")
    x_dw = nc.dram_tensor("x_dw", [B, C, DSH, SMALL, FULL], BF16, kind="ExternalInput")
    x_hw = nc.dram_tensor("x_hw", [B, C, SMALL, DSH, FULL], BF16, kind="ExternalInput")
    # one packed bf16 const tensor per core: block-diag transposed weights,
    # pooled summary tables, bf16 bias (see _CST_* offsets)
    cst = nc.dram_tensor("cst", [P, CLEN], BF16, kind="ExternalInput")

    y_dh = nc.dram_tensor("y_dh", [B, C, DSH, FULL, SMALL], BF16, kind="ExternalOutput")
    y_dw = nc.dram_tensor("y_dw", [B, C, DSH, SMALL, FULL], BF16, kind="ExternalOutput")
    y_hw = nc.dram_tensor("y_hw", [B, C, SMALL, DSH, FULL], BF16, kind="ExternalOutput")

    with tile.TileContext(nc) as tc, ExitStack() as ctx:
        cpool = ctx.enter_context(tc.tile_pool(name="const", bufs=1))
        inp = ctx.enter_context(tc.tile_pool(name="inp", bufs=4))
        psp = ctx.enter_context(tc.tile_pool(name="psum", bufs=2, space="PSUM"))
        outp = ctx.enter_context(tc.tile_pool(name="outp", bufs=4))

        # ---- all constants in ONE packed DMA (contiguous lines) ----
        csb = cpool.tile([P, CLEN], BF16)
        nc.sync.dma_start(csb[:], cst.ap())
        l1sb = csb[:, OFF_L1 : OFF_L1 + DSH * SMALL].rearrange(
            "p (a b) -> p a b", a=DSH
        )
        l2sb = csb[:, OFF_L2 : OFF_L2 + DSH * SMALL].rearrange(
            "p (a b) -> p a b", a=DSH
        )
        g1sb = csb[:, OFF_G1 : OFF_G1 + FULL * SMALL].rearrange(
            "p (a b) -> p a b", a=FULL
        )
        g2sb = csb[:, OFF_G2 : OFF_G2 + SMALL * FULL].rearrange(
            "p (a b) -> p a b", a=SMALL
        )
        g3sb = csb[:, OFF_G3 : OFF_G3 + SMALL * DSH].rearrange(
            "p (a b) -> p a b", a=SMALL
        )
        g4sb = csb[:, OFF_G4 : OFF_G4 + SMALL * FULL].rearrange(
            "p (a b) -> p a b", a=SMALL
        )
        bsb = cpool.tile([P, 3], F32)
        nc.vector.tensor_copy(bsb[:], csb[:, OFF_B : OFF_B + 3])

        def w_of(plane, s):
            i = (3 * plane + s) * P
            return csb[:, i : i + P]

        def evict_half(plane, ps_ap, osb_ap, on_act):
            # PSUM -> SBUF with per-channel bias, fp32 -> bf16.
            if on_act:
                nc.scalar.activation(
                    osb_ap, ps_ap, mybir.ActivationFunctionType.Identity,
                    bias=bsb[:, plane : plane + 1],
                )
            else:
                nc.vector.tensor_scalar_add(osb_ap, ps_ap, bsb[:, plane : plane + 1])

        # ---------- planes 0/1: dh, dw (identical structure) ----------
        # in tile = d-quad [128, 4, 1024] (8KB contiguous lines); 2 per plane
        for plane, x, y in ((0, x_dh, y_dh), (1, x_dw, y_dw)):
            xv = x.ap().rearrange("b c d h w -> (b c) d (h w)")
            yv = y.ap().rearrange("b c d h w -> (b c) (d h w)")
            for q in range(DSH // 4):
                tin = inp.tile([P, 4, 1024], BF16, tag="in")
                nc.sync.dma_start(tin[:], xv[:, 4 * q : 4 * q + 4, :])
                for t in range(2):  # psum tile per d-pair
                    ps = psp.tile([P, 2048], F32)
                    for s in range(3):
                        lhsT = w_of(plane, s)
                        for j in range(2):  # d within pair
                            dl = 4 * q + 2 * t + j
                            for n in range(2):  # 512-chunk
                                if s == 0:
                                    rhs = tin[:, 2 * t + j, NCH * n : NCH * (n + 1)]
                                elif s == 1:
                                    if plane == 0:
                                        # l1[dl, w16] bcast over h (chunk rows)
                                        rhs = l1sb[:, dl : dl + 1, :].broadcast_to(
                                            [P, 32, SMALL]
                                        )
                                    else:
                                        # l2[dl, h16] chunk, bcast over w
                                        rhs = (
                                            l2sb[:, dl, 8 * n : 8 * (n + 1)]
                                            .unsqueeze(2)
                                            .broadcast_to([P, 8, FULL])
                                        )
                                elif plane == 0:
                                    rhs = g1sb[:, 32 * n : 32 * (n + 1), :]
                                else:
                                    rhs = g2sb[:, 8 * n : 8 * (n + 1), :]
                                nc.tensor.matmul(
                                    ps[:, (j * 2 + n) * NCH : (j * 2 + n + 1) * NCH],
                                    lhsT,
                                    rhs,
                                    start=(s == 0),
                                    stop=(s == 2),
                                )
                    osb = outp.tile([P, 2048], BF16, tag="out")
                    evict_half(plane, ps[:], osb[:], on_act=False)
                    nc.scalar.dma_start(
                        yv[:, (4 * q + 2 * t) * 1024 : (4 * q + 2 * t + 2) * 1024],
                        osb[:],
                    )

        # ---------- plane 2: hw ----------
        # in tile = h-quad [128, 16, 4, 64] (512B lines); 2 tiles
        xv = x_hw.ap().rearrange("b c d h w -> (b c) d h w")
        yv = y_hw.ap().rearrange("b c d h w -> (b c) d h w")
        for q in range(DSH // 4):
            tin = inp.tile([P, SMALL, 4, FULL], BF16, tag="in")
            nc.sync.dma_start(tin[:], xv[:, :, 4 * q : 4 * q + 4, :])
            for t in range(2):  # psum tile per 8 d's
                ps = psp.tile([P, 2048], F32)
                for s in range(3):
                    lhsT = w_of(2, s)
                    for m in range(4):  # chunk: d in [8t+2m, 8t+2m+2)
                        d0 = 8 * t + 2 * m
                        if s == 0:
                            rhs = tin[:, d0 : d0 + 2, :, :]
                        elif s == 1:
                            # g3[d16, h_loc] chunk, bcast over w
                            rhs = (
                                g3sb[:, d0 : d0 + 2, 4 * q : 4 * q + 4]
                                .unsqueeze(3)
                                .broadcast_to([P, 2, 4, FULL])
                            )
                        else:
                            # g4[d16, w] chunk, bcast over h (middle)
                            rhs = (
                                g4sb[:, d0 : d0 + 2, :]
                                .unsqueeze(2)
                                .broadcast_to([P, 2, 4, FULL])
                            )
                        nc.tensor.matmul(
                            ps[:, m * NCH : (m + 1) * NCH],
                            lhsT,
                            rhs,
                            start=(s == 0),
                            stop=(s == 2),
                        )
                osb = outp.tile([P, 8, 4, FULL], BF16, tag="out")
                osb_flat = osb[:].rearrange("p a b c -> p (a b c)")
                if q == DSH // 4 - 1 and t == 1:
                    # final tile: halves on DVE||ACT + 2 DMAs to cut the tail
                    for j in range(2):
                        h = slice(j * 1024, (j + 1) * 1024)
                        evict_half(2, ps[:, h], osb_flat[:, h], on_act=(j == 1))
                        nc.scalar.dma_start(
                            yv[:, 8 * t + 4 * j : 8 * t + 4 * (j + 1), 4 * q : 4 * q + 4, :],
                            osb[:, 4 * j : 4 * (j + 1)],
                        )
                else:
                    evict_half(2, ps[:], osb_flat, on_act=False)
                    nc.scalar.dma_start(
                        yv[:, 8 * t : 8 * t + 8, 4 * q : 4 * q + 4, :], osb[:]
                    )

    nc.compile()
    return nc


def _pool4(x, axis):
    # exact adaptive mean-pool by 4 along `axis` (64 -> 16)
    shp = list(x.shape)
    shp[axis] = 16
    shp.insert(axis + 1, 4)
    return x.reshape(shp).mean(axis=axis + 1)


def _prep_inputs(dh, dw, hw, w_dh, b_dh, w_dw, b_dw, w_hw, b_hw):
    f32 = np.float32
    dh, dw, hw = (np.ascontiguousarray(a, f32) for a in (dh, dw, hw))

    sum_dh = dh.mean(axis=4)  # [b,c,d,h]
    sum_dw = dw.mean(axis=3)  # [b,c,d,w]
    sum_hw = hw.mean(axis=2)  # [b,c,h,w]

    p_wdw = _pool4(sum_dw, 3)  # [b,c,d,16]   dw_in_dh
    p_hdh = _pool4(sum_dh, 3)  # [b,c,d,16]   dh_in_dw
    p_whw = _pool4(sum_hw, 3)  # [b,c,h,16]   hw_in_dh
    p_hhw = _pool4(sum_hw, 2)  # [b,c,16,w]   hw_in_dw
    p_ddh = _pool4(sum_dh, 2)  # [b,c,16,h]   dh_in_hw
    p_ddw = _pool4(sum_dw, 2)  # [b,c,16,w]   dw_in_hw

    bf = lambda a: np.ascontiguousarray(a, NPBF16)
    dh, dw, hw = bf(dh), bf(dw), bf(hw)

    # packed per-core const tensor [P, CLEN]
    cst0 = np.zeros((P, CLEN), NPBF16)
    for pi, w in enumerate((w_dh, w_dw, w_hw)):
        w = np.asarray(w, f32)
        for s in range(3):
            blk = bf(w[:, 64 * s : 64 * (s + 1)].T)  # [c_in, o]
            i = (3 * pi + s) * P
            cst0[0:64, i : i + 64] = blk
            cst0[64:128, i + 64 : i + 128] = blk
    cst0[:, OFF_B : OFF_B + 3] = bf(
        np.stack(
            [np.concatenate([np.asarray(b, f32)] * 2) for b in (b_dh, b_dw, b_hw)],
            axis=1,
        )
    )
    cst0[:, OFF_G1 : OFF_G1 + FULL * SMALL] = bf(p_whw.reshape(P, FULL * SMALL))
    cst0[:, OFF_G2 : OFF_G2 + SMALL * FULL] = bf(p_hhw.reshape(P, SMALL * FULL))
    cst0[:, OFF_G4 : OFF_G4 + SMALL * FULL] = bf(p_ddw.reshape(P, SMALL * FULL))

    in_maps = []
    for k in range(NCORES):
        dsl = slice(DSH * k, DSH * (k + 1))
        cst = cst0.copy()
        cst[:, OFF_G3 : OFF_G3 + SMALL * DSH] = bf(
            p_ddh.reshape(P, SMALL, FULL)[:, :, dsl].reshape(P, SMALL * DSH)
        )
        cst[:, OFF_L1 : OFF_L1 + DSH * SMALL] = bf(
            p_wdw.reshape(P, FULL, SMALL)[:, dsl, :].reshape(P, DSH * SMALL)
        )
        cst[:, OFF_L2 : OFF_L2 + DSH * SMALL] = bf(
            p_hdh.reshape(P, FULL, SMALL)[:, dsl, :].reshape(P, DSH * SMALL)
        )
        in_maps.append(
            {
                "x_dh": np.ascontiguousarray(dh[:, :, dsl]),
                "x_dw": np.ascontiguousarray(dw[:, :, dsl]),
                "x_hw": np.ascontiguousarray(hw[:, :, :, dsl, :]),
                "cst": cst,
            }
        )
    return in_maps


def _run(inputs: dict, trace: bool = False):
    global _BUILT, LAST_RESULTS
    if _BUILT is None:
        _BUILT = _build()
    nc = _BUILT

    in_maps = _prep_inputs(**inputs)
    res = bass_utils.run_bass_kernel_spmd(
        nc, in_maps, core_ids=list(range(NCORES)), trace=trace
    )
    LAST_RESULTS = res

    dh_new = np.empty((B, C, FULL, FULL, SMALL), np.float32)
    dw_new = np.empty((B, C, FULL, SMALL, FULL), np.float32)
    hw_new = np.empty((B, C, SMALL, FULL, FULL), np.float32)
    for k in range(NCORES):
        dsl = slice(DSH * k, DSH * (k + 1))
        dh_new[:, :, dsl] = res.results[k]["y_dh"].astype(np.float32)
        dw_new[:, :, dsl] = res.results[k]["y_dw"].astype(np.float32)
        hw_new[:, :, :, dsl, :] = res.results[k]["y_hw"].astype(np.float32)
    return dh_new, dw_new, hw_new


def kernel(**inputs):
    return _run(inputs, trace=bool(os.environ.get("KERNEL_TRACE")))
